# revision 5
# baseline (speedup 1.0000x reference)
"""Distributed Trainium2 Bass kernel for the A2C GNN message-passing model.

Strategy (8 NeuronCores, node-row sharding, 512 rows/core):
  - SAGE aggregation as dense-adjacency matmuls on TensorE: the host builds
    AT[j,i] = count(src=j -> dst=i) / max(indeg(i),1) once; each core keeps its
    512-column slice in SBUF (bf16) and computes agg_T[f, own_i] = sum_k
    h_nat[k-chunk] @ AT[k-chunk] (32 accumulating matmuls, f32 PSUM).
  - Feature maps are kept transposed ([feat, node]) so SAGE biases are
    per-partition activation biases; tanh fused into the PSUM->SBUF copy.
  - After layers 1 and 2 the 512 new rows are AllGathered (bf16, 128KB/rank)
    to rebuild the full natural-layout h for the next aggregation.
  - Actor and critic heads share the layer-3 aggregation. Only the projected
    scalars a = Xa@w1, b = Xa@w2, cp = Xc@wfc are AllGathered (f32, 6KB).
  - The N^2 log_softmax factorizes: out[i,j] = a_i + b_j + bfa - LSE with
    LSE = bfa + logsumexp(a) + logsumexp(b), so bfa cancels and
    out[i,j] = b_j + (a_i - La - Lb). Each core writes its 512x4096 f32 block:
    partition-broadcast of the b row + one tensor_scalar add per 128-row tile.
  - edge_critic = tanh(mean(cp) + bfc) (the (cp_i+cp_j)/2 mean collapses).
"""

import numpy as np
import ml_dtypes

N = 4096
U = 128
NCORES = 8
R = N // NCORES          # 512 rows per core
KT = N // 128            # 32 contraction chunks
RC = R // 128            # 4 row chunks per core

bf16 = np.float16  # fp16: same PE rate as bf16, 8x finer mantissa

_STATE = {}


def _build_nc():
    import concourse.bass as bass
    import concourse.bacc as bacc
    import concourse.mybir as mybir
    import concourse.tile as tile

    f32 = mybir.dt.float32
    bf = mybir.dt.float16
    AX = mybir.AxisListType.X
    AF = mybir.ActivationFunctionType

    nc = bacc.Bacc("TRN2", target_bir_lowering=False, debug=False,
                   num_devices=NCORES)

    # ---- kernel I/O ----
    amat = nc.dram_tensor("amat", [KT, 128, R], bf, kind="ExternalInput")
    xnat = nc.dram_tensor("xnat", [KT, 128, U], bf, kind="ExternalInput")
    xt = nc.dram_tensor("xt", [U, R], bf, kind="ExternalInput")
    wts = nc.dram_tensor("wts", [8, U, U], bf, kind="ExternalInput")
    biases = nc.dram_tensor("biases", [U, 4], f32, kind="ExternalInput")
    w12 = nc.dram_tensor("w12", [U, 2], bf, kind="ExternalInput")
    wfc = nc.dram_tensor("wfc", [U, 1], bf, kind="ExternalInput")
    bfc = nc.dram_tensor("bfc", [1, 1], f32, kind="ExternalInput")
    iden = nc.dram_tensor("iden", [128, 128], bf, kind="ExternalInput")
    ones1 = nc.dram_tensor("ones1", [1, 1], f32, kind="ExternalInput")
    ones128 = nc.dram_tensor("ones128", [128, 1], f32, kind="ExternalInput")

    out = nc.dram_tensor("out", [R, N], f32, kind="ExternalOutput")
    crit = nc.dram_tensor("crit", [1, 1], f32, kind="ExternalOutput")

    # ---- collective bounce buffers (internal DRAM) ----
    ag_in = [nc.dram_tensor(f"ag{l}_in", [RC, 128, U], bf) for l in (1, 2)]
    ag_out = [nc.dram_tensor(f"ag{l}_out", [KT, 128, U], bf,
                             addr_space="Shared") for l in (1, 2)]
    ag3_in = nc.dram_tensor("ag3_in", [3, R], f32)
    ag3_out = nc.dram_tensor("ag3_out", [NCORES, 3, R], f32,
                             addr_space="Shared")
    rgroups = [list(range(NCORES))]

    with tile.TileContext(nc) as tc:
        with tc.tile_pool(name="const", bufs=1) as cst, \
             tc.tile_pool(name="work", bufs=1) as wrk, \
             tc.tile_pool(name="big", bufs=1) as big, \
             tc.tile_pool(name="opool", bufs=2) as opool, \
             tc.tile_pool(name="pag", bufs=1, space="PSUM") as pagp, \
             tc.tile_pool(name="pw", bufs=1, space="PSUM") as pwp, \
             tc.tile_pool(name="pt", bufs=1, space="PSUM") as ptp, \
             tc.tile_pool(name="psm", bufs=2, space="PSUM") as psm:

            # ---- constant loads ----
            amat_sb = cst.tile([128, KT * R], bf, tag="amat")
            for k in range(KT):
                nc.sync.dma_start(amat_sb[:, k * R:(k + 1) * R], amat[k])
            xnat_sb = cst.tile([128, KT * U], bf, tag="xnat")
            for k0 in range(0, KT, 8):
                nc.sync.dma_start(
                    xnat_sb[:, k0 * U:(k0 + 8) * U].rearrange(
                        "p (k f) -> p k f", k=8),
                    xnat[k0:k0 + 8].rearrange("k p f -> p k f"))
            xt_sb = cst.tile([128, R], bf, tag="xt")
            nc.sync.dma_start(xt_sb[:], xt[:, :])
            wts_sb = cst.tile([128, 8 * U], bf, tag="wts")
            for w in range(8):
                nc.sync.dma_start(wts_sb[:, w * U:(w + 1) * U], wts[w])
            bias_sb = cst.tile([128, 4], f32, tag="bias")
            nc.sync.dma_start(bias_sb[:], biases[:, :])
            w12_sb = cst.tile([128, 2], bf, tag="w12")
            nc.sync.dma_start(w12_sb[:], w12[:, :])
            wfc_sb = cst.tile([128, 1], bf, tag="wfc")
            nc.sync.dma_start(wfc_sb[:], wfc[:, :])
            bfc_sb = cst.tile([1, 1], f32, tag="bfc")
            nc.sync.dma_start(bfc_sb[:], bfc[:, :])
            iden_sb = cst.tile([128, 128], bf, tag="iden")
            nc.sync.dma_start(iden_sb[:], iden[:, :])
            one1_sb = cst.tile([1, 1], f32, tag="one1")
            nc.sync.dma_start(one1_sb[:], ones1[:, :])
            one128_sb = cst.tile([128, 1], f32, tag="one128")
            nc.sync.dma_start(one128_sb[:], ones128[:, :])

            hnat = [None, None]   # full natural h (layers 1, 2)
            hT = [None, None]     # transposed own-columns h

            def sage_agg(lhs_sb):
                """agg_T[f, own_i] accumulated over 32 k-chunk matmuls."""
                pag = pagp.tile([128, R], f32, tag="pag")
                for k in range(KT):
                    nc.tensor.matmul(pag[:], lhs_sb[:, k * U:(k + 1) * U],
                                     amat_sb[:, k * R:(k + 1) * R],
                                     start=(k == 0), stop=(k == KT - 1))
                aggT = wrk.tile([128, R], bf, tag="aggT")
                nc.scalar.copy(aggT[:], pag[:])
                return aggT

            # ================= layers 1 and 2 =================
            for l in range(2):
                lhs = xnat_sb if l == 0 else hnat[0]
                rhsT = xt_sb if l == 0 else hT[0]
                aggT = sage_agg(lhs)
                ph = pwp.tile([128, R], f32, tag="ph")
                nc.tensor.matmul(ph[:], wts_sb[:, (2 * l) * U:(2 * l + 1) * U],
                                 aggT[:], start=True, stop=False)
                nc.tensor.matmul(ph[:], wts_sb[:, (2 * l + 1) * U:(2 * l + 2) * U],
                                 rhsT[:], start=False, stop=True)
                hT_new = wrk.tile([128, R], bf, tag=f"hT{l}")
                nc.scalar.activation(hT_new[:], ph[:], AF.Tanh,
                                     bias=bias_sb[:, l:l + 1])
                hT[l] = hT_new
                # transpose own columns back to natural layout
                pt = ptp.tile([128, R], bf, tag="pt")
                for c in range(RC):
                    nc.tensor.transpose(pt[:, c * 128:(c + 1) * 128],
                                        hT_new[:, c * 128:(c + 1) * 128],
                                        iden_sb[:])
                hc = wrk.tile([128, R], bf, tag=f"hc{l}")
                nc.scalar.copy(hc[:], pt[:])
                nc.sync.dma_start(
                    ag_in[l].ap().rearrange("c p f -> p c f"),
                    hc[:].rearrange("p (c f) -> p c f", c=RC))
                nc.gpsimd.collective_compute(
                    "AllGather", mybir.AluOpType.bypass,
                    ins=[ag_in[l].ap().opt()], outs=[ag_out[l].ap().opt()],
                    replica_groups=rgroups)
                hn = big.tile([128, KT * U], bf, tag=f"hnat{l}")
                for k0 in range(0, KT, 8):
                    nc.sync.dma_start(
                        hn[:, k0 * U:(k0 + 8) * U].rearrange(
                            "p (k f) -> p k f", k=8),
                        ag_out[l][k0:k0 + 8].rearrange("k p f -> p k f"))
                hnat[l] = hn

            # ================= heads (shared aggregation) =================
            aggT3 = sage_agg(hnat[1])
            headT = []
            for hi, (wl, wr, bcol) in enumerate(((4, 5, 2), (6, 7, 3))):
                ph = pwp.tile([128, R], f32, tag="ph")
                nc.tensor.matmul(ph[:], wts_sb[:, wl * U:(wl + 1) * U],
                                 aggT3[:], start=True, stop=False)
                nc.tensor.matmul(ph[:], wts_sb[:, wr * U:(wr + 1) * U],
                                 hT[1][:], start=False, stop=True)
                xh = wrk.tile([128, R], bf, tag=f"headT{hi}")
                nc.vector.tensor_scalar_add(xh[:], ph[:], bias_sb[:, bcol:bcol + 1])
                headT.append(xh)

            pab = psm.tile([2, R], f32, tag="small")
            nc.tensor.matmul(pab[:], w12_sb[:], headT[0][:], start=True, stop=True)
            pcp = psm.tile([1, R], f32, tag="small")
            nc.tensor.matmul(pcp[:], wfc_sb[:], headT[1][:], start=True, stop=True)
            abc_own = wrk.tile([2, R], f32, tag="abc_own")
            nc.scalar.copy(abc_own[:], pab[:])
            cp_own = wrk.tile([1, R], f32, tag="cp_own")
            nc.scalar.copy(cp_own[:], pcp[:])
            nc.sync.dma_start(ag3_in[0:2, :], abc_own[:])
            nc.sync.dma_start(ag3_in[2, :], cp_own[:])
            nc.gpsimd.collective_compute(
                "AllGather", mybir.AluOpType.bypass,
                ins=[ag3_in.ap().opt()], outs=[ag3_out.ap().opt()],
                replica_groups=rgroups)

            # ---- global logsumexp of a and b (parallel [128, 32] layout) ----
            abcpm = wrk.tile([128, 3 * 32], f32, tag="abcpm")
            for t in range(3):
                nc.sync.dma_start(abcpm[:, t * 32:(t + 1) * 32],
                                  ag3_out[:, t, :])
            b_row = wrk.tile([1, N], f32, tag="b_row")
            nc.sync.dma_start(b_row[:], ag3_out[:, 1, :])

            Ls = []
            for t in range(2):
                v = abcpm[:, t * 32:(t + 1) * 32]
                negm = wrk.tile([128, 1], f32, tag=f"negm{t}")
                nc.vector.reduce_max(negm[:], v, axis=AX, negate=True)
                e = wrk.tile([128, 32], f32, tag=f"e{t}")
                es = wrk.tile([128, 1], f32, tag=f"es{t}")
                nc.scalar.activation(e[:], v, AF.Exp, bias=negm[:, 0:1],
                                     accum_out=es[:, 0:1])
                emp = wrk.tile([128, 1], f32, tag=f"emp{t}")
                nc.scalar.activation(emp[:], negm[:], AF.Exp, scale=-1.0)
                sp = wrk.tile([128, 1], f32, tag=f"sp{t}")
                nc.vector.tensor_tensor(sp[:], es[:], emp[:],
                                        op=mybir.AluOpType.mult)
                ptot = psm.tile([1, 1], f32, tag="small")
                nc.tensor.matmul(ptot[:], one128_sb[:], sp[:],
                                 start=True, stop=True)
                L = wrk.tile([1, 1], f32, tag=f"L{t}")
                nc.scalar.activation(L[:], ptot[:], AF.Ln)
                Ls.append(L)

            negL2 = wrk.tile([1, 1], f32, tag="negL2")
            nc.vector.tensor_tensor(negL2[:], Ls[0][:], Ls[1][:],
                                    op=mybir.AluOpType.add)
            nc.scalar.mul(negL2[:], negL2[:], -1.0)

            # critic: tanh(sum(cp)/N + bfc)
            cps = wrk.tile([128, 1], f32, tag="cps")
            nc.vector.reduce_sum(cps[:], abcpm[:, 64:96], axis=AX)
            pct = psm.tile([1, 1], f32, tag="small")
            nc.tensor.matmul(pct[:], one128_sb[:], cps[:], start=True, stop=True)
            crit_sb = wrk.tile([1, 1], f32, tag="crit_sb")
            nc.scalar.activation(crit_sb[:], pct[:], AF.Tanh,
                                 scale=1.0 / N, bias=bfc_sb[0:1, 0:1])
            nc.sync.dma_start(crit[:, :], crit_sb[:])

            # alpha[i] = a_own[i] - La - Lb, moved to partition axis
            arow = wrk.tile([1, R], f32, tag="arow")
            nc.vector.tensor_scalar_add(arow[:], abc_own[0:1, :],
                                        negL2[0:1, 0:1])
            pa = psm.tile([128, RC], f32, tag="small")
            for c in range(RC):
                nc.tensor.matmul(pa[:, c:c + 1],
                                 arow[0:1, c * 128:(c + 1) * 128],
                                 one1_sb[:], start=True, stop=True)
            alpha = wrk.tile([128, RC], f32, tag="alpha")
            nc.scalar.copy(alpha[:], pa[:])

            # ---- the big output: out[i, j] = b[j] + alpha[i] ----
            bb = big.tile([128, N], f32, tag="bb")
            nc.gpsimd.partition_broadcast(bb[:], b_row[:])
            for c in range(RC):
                ob = opool.tile([128, N], f32, tag="ob")
                nc.vector.tensor_scalar_add(ob[:], bb[:], alpha[:, c:c + 1])
                for h in range(2):
                    nc.sync.dma_start(
                        out[c * 128:(c + 1) * 128, h * 2048:(h + 1) * 2048],
                        ob[:, h * 2048:(h + 1) * 2048])

    nc.compile()
    return nc


def _get_nc():
    if "nc" not in _STATE:
        import concourse.bass as bass  # noqa: F401
        _STATE["nc"] = _build_nc()
    return _STATE["nc"]


def _host_prep(inputs):
    x = np.asarray(inputs["x"], np.float32)
    ei = np.asarray(inputs["edge_index"])
    src = ei[0].astype(np.int64)
    dst = ei[1].astype(np.int64)

    AT = np.zeros((N, N), np.float32)
    np.add.at(AT, (src, dst), 1.0)
    deg = np.bincount(dst, minlength=N).astype(np.float32)
    ATn = AT / np.maximum(deg, 1.0)[None, :]

    wts = np.stack([
        inputs["Wf_l"].T, inputs["Wf_r"].T,
        inputs["Wcm_l"].T, inputs["Wcm_r"].T,
        inputs["Wa_l"].T, inputs["Wa_r"].T,
        inputs["Wcr_l"].T, inputs["Wcr_r"].T,
    ]).astype(bf16)
    biases = np.stack([
        inputs["bf_l"], inputs["bcm_l"], inputs["ba_l"], inputs["bcr_l"],
    ], axis=1).astype(np.float32)
    Wfa = np.asarray(inputs["Wfa"], np.float32)
    w12 = np.stack([Wfa[0, :U], Wfa[0, U:]], axis=1).astype(bf16)
    wfc = np.asarray(inputs["Wfc"], np.float32)[0][:, None].astype(bf16)
    bfc = np.asarray(inputs["bfc"], np.float32).reshape(1, 1)

    common = {
        "xnat": np.ascontiguousarray(x.astype(bf16).reshape(KT, 128, U)),
        "wts": wts,
        "biases": biases,
        "w12": w12,
        "wfc": wfc,
        "bfc": bfc,
        "iden": np.eye(128, dtype=bf16),
        "ones1": np.ones((1, 1), np.float32),
        "ones128": np.ones((128, 1), np.float32),
    }
    in_maps = []
    for c in range(NCORES):
        sl = slice(c * R, (c + 1) * R)
        m = dict(common)
        m["amat"] = np.ascontiguousarray(
            ATn[:, sl].astype(bf16).reshape(KT, 128, R))
        m["xt"] = np.ascontiguousarray(x[sl].T.astype(bf16))
        in_maps.append(m)
    return in_maps


def _run(inputs, trace=False):
    from concourse.bass_utils import run_bass_kernel_spmd
    nc = _get_nc()
    in_maps = _host_prep(inputs)
    res = run_bass_kernel_spmd(nc, in_maps, core_ids=list(range(NCORES)),
                               trace=trace)
    edge_actor = np.concatenate(
        [np.asarray(res.results[c]["out"], np.float32) for c in range(NCORES)],
        axis=0).reshape(N * N, 1)
    edge_critic = np.asarray(res.results[0]["crit"], np.float32).reshape(1, 1)
    return (edge_actor, edge_critic), res


def kernel(**inputs):
    outputs, _ = _run(inputs, trace=False)
    return outputs
